# revision 14
# baseline (speedup 1.0000x reference)
"""Causal self-attention on 8 trn2 NeuronCores.

Full inputs in, full output out. Sharding: data-parallel over batch (B=4),
tensor-parallel over head groups (16 heads -> 2 groups of 8). core = 2*b + g.

Per-core math (T=2048, C=1024, 8 heads, D=64, group channels G=512):
  qT/kT: [64*(h%2)+d, h//2, t] layout so scores need no transposes
  scoresT[j,i] = sum_d kT[d,j] qT[d,i]   (q pre-scaled by 1/sqrt(D) on host)
  softmax without max-subtraction (scores ~ N(0,1) by construction; exp is
  exactly shift-invariant so this matches the reference softmax)
  expT row sums come free from an all-ones 65th column appended to V
  causal mask via affine_select (fill 0 post-exp) on diagonal blocks only
  out_T[d,i] = sum_j v[j,d] expT[j,i]; normalize by 1/sums; y = oT.T @ Wp

Host gather: y[b] = part[2b] + part[2b+1] + b_attn_v @ W_proj + b_proj
(q/k biases are added on-device; the v bias commutes through softmax).

All matmul inputs are float32r (hardware rounds fp32 on read; measured
~1.5e-4 rel err per K=1024 matmul, at full 1 cyc/row PE rate for N>=256).
"""

import numpy as np
from contextlib import ExitStack

import concourse.bass as bass
import concourse.tile as tile
from concourse import bacc, mybir
from concourse.bass_utils import run_bass_kernel_spmd

P = 128
B, T, C, H = 4, 2048, 1024, 16
D = 64
HG = 8          # heads per core
G = HG * D      # 512 head channels per core
CT = C // P     # 8 contraction tiles
TCH = T // 512  # 4 chunks of 512 tokens
NT = G // P     # 4 tiles of head channels

f32 = mybir.dt.float32
f32r = mybir.dt.float32r
bf16 = mybir.dt.bfloat16
MM_DT = bf16    # dtype of all matmul inputs (bf16: FWL weight loads, 1 cyc/row)
EXP_DT = MM_DT  # dtype of exp(scores) tiles (step-4 rhs)


def build_attention(nc: bass.Bass):
    xT = nc.dram_tensor("xT", [C, T], MM_DT, kind="ExternalInput")
    wq = nc.dram_tensor("wq", [C, G], MM_DT, kind="ExternalInput")
    wk = nc.dram_tensor("wk", [C, G], MM_DT, kind="ExternalInput")
    wv = nc.dram_tensor("wv", [C, G], MM_DT, kind="ExternalInput")
    wp = nc.dram_tensor("wp", [G, C], MM_DT, kind="ExternalInput")
    bq = nc.dram_tensor("bq", [P, NT], f32, kind="ExternalInput")
    bk = nc.dram_tensor("bk", [P, NT], f32, kind="ExternalInput")
    y = nc.dram_tensor("y", [T, C], f32, kind="ExternalOutput")

    with tile.TileContext(nc) as tc, ExitStack() as ctx:
        persist = ctx.enter_context(tc.tile_pool(name="persist", bufs=1))
        qT = persist.tile([P, NT, T], MM_DT)
        kT = persist.tile([P, NT, T], MM_DT)
        v_aug = persist.tile([P, T // P, HG, D + 1], MM_DT)
        wp_sb = persist.tile([P, NT, C], MM_DT)
        bq_sb = persist.tile([P, NT], f32)
        bk_sb = persist.tile([P, NT], f32)

        for nt in range(NT):
            nc.sync.dma_start(out=wp_sb[:, nt, :], in_=wp.ap()[P * nt:P * (nt + 1), :])
        nc.sync.dma_start(out=bq_sb, in_=bq.ap())
        nc.sync.dma_start(out=bk_sb, in_=bk.ap())
        ones_col = persist.tile([P, 1], f32)
        nc.vector.memset(ones_col, 1.0)
        nc.vector.tensor_copy(
            out=v_aug[:, :, :, D:D + 1],
            in_=ones_col.to_broadcast([P, T // P, HG, 1]),
        )

        # ---------------- phase 1: QKV projections ----------------
        with (
            tc.tile_pool(name="wpool", bufs=1) as wpool,
            tc.tile_pool(name="xpool", bufs=10) as xpool,
            tc.tile_pool(name="ps1", bufs=3, space="PSUM") as ps1,
        ):
            wq_sb = wpool.tile([P, CT, G], MM_DT, tag="wq")
            wk_sb = wpool.tile([P, CT, G], MM_DT, tag="wk")
            wv_sb = wpool.tile([P, CT, G], MM_DT, tag="wv")
            for ct in range(CT):
                nc.sync.dma_start(out=wq_sb[:, ct, :], in_=wq.ap()[P * ct:P * (ct + 1), :])
                nc.sync.dma_start(out=wk_sb[:, ct, :], in_=wk.ap()[P * ct:P * (ct + 1), :])
                nc.sync.dma_start(out=wv_sb[:, ct, :], in_=wv.ap()[P * ct:P * (ct + 1), :])

            for tch in range(TCH):
                xts = []
                for ct in range(CT):
                    xt = xpool.tile([P, 512], MM_DT, tag="xt", name=f"xt_{tch}_{ct}")
                    nc.sync.dma_start(
                        out=xt, in_=xT.ap()[P * ct:P * (ct + 1), 512 * tch:512 * (tch + 1)]
                    )
                    xts.append(xt)
                # qT, kT:  psum[j,t] += wq[c,j].T @ xT[c,t]
                for w_sb, b_sb, dstT in ((wq_sb, bq_sb, qT), (wk_sb, bk_sb, kT)):
                    for jt in range(NT):
                        ps = ps1.tile([P, 512], f32, tag="pqk", name=f"pqk_{tch}_{jt}")
                        for ct in range(CT):
                            nc.tensor.matmul(
                                ps, w_sb[:, ct, P * jt:P * (jt + 1)], xts[ct],
                                start=(ct == 0), stop=(ct == CT - 1),
                            )
                        nc.vector.tensor_scalar_add(
                            out=dstT[:, jt, 512 * tch:512 * (tch + 1)],
                            in0=ps, scalar1=b_sb[:, jt:jt + 1],
                        )
                # v: psum[t,jv] += xT[c,t-tile].T @ wv[c,jv]
                for tt4 in range(4):
                    tt = 4 * tch + tt4
                    ps = ps1.tile([P, 512], f32, tag="pv", name=f"pv_{tt}")
                    for ct in range(CT):
                        nc.tensor.matmul(
                            ps, xts[ct][:, P * tt4:P * (tt4 + 1)], wv_sb[:, ct, :],
                            start=(ct == 0), stop=(ct == CT - 1),
                        )
                    nc.vector.tensor_copy(
                        out=v_aug[:, tt, :, 0:D],
                        in_=ps.rearrange("p (h d) -> p h d", h=HG),
                    )

        # ---------------- phase 2: attention ----------------
        # Packed scores: 2 heads concurrently on the PE via tile_position
        # 4-tile row+col packing (head h occupies array rows 64*(h%2)..+64).
        # Variable-width trapezoid tiles: block (jb, ic) only covers queries
        # i >= 128*jb, so all of scores/exp/out use columns [off:] with
        # off = max(0, 128*jb - 512*ic). Diagonal 128-wide triangle masked
        # with a single [128,128] affine_select.
        opool = ctx.enter_context(tc.tile_pool(name="opool", bufs=1))
        oT = opool.tile([P, NT, T], MM_DT)
        with (
            tc.tile_pool(name="epool", bufs=6) as epool,
            tc.tile_pool(name="upool", bufs=10) as upool,
            tc.tile_pool(name="spool", bufs=2) as spool,
            tc.tile_pool(name="bpool", bufs=3) as bpool,
            tc.tile_pool(name="ps_s", bufs=2, space="PSUM") as ps_s,
            tc.tile_pool(name="ps_o", bufs=3, space="PSUM") as ps_o,
        ):
            for g2 in range(HG // 2):  # head pairs share nt = g2
                S_pair = spool.tile([8, 512], f32, tag="S", name=f"S_{g2}")
                o_us = {}
                for ic in range(TCH):
                    o_ps = {}
                    for hh in range(2):
                        o_ps[hh] = ps_o.tile([D + 1, 512], f32, tag="o",
                                             name=f"ops_{2 * g2 + hh}_{ic}")
                    n_jb = 4 * ic + 4
                    for jb in range(n_jb):
                        off = max(0, P * jb - 512 * ic)
                        w = 512 - off
                        # both heads' scores in one 2-bank psum tile:
                        # cols [off:512] = h0 (from array rows 0:64),
                        # cols [512+off:1024] = h1 (rows 64:128)
                        s_big = ps_s.tile([P, 1024], f32, tag="s",
                                          name=f"sps_{g2}_{ic}_{jb}")
                        with tc.tile_critical():
                            for hh in range(2):
                                band = 64 * hh
                                for cg in range(2):
                                    nc.tensor.matmul(
                                        s_big[64 * cg:64 * cg + 64,
                                              512 * hh + off:512 * (hh + 1)],
                                        kT[band:band + D, g2,
                                           P * jb + 64 * cg:P * jb + 64 * cg + 64],
                                        qT[band:band + D, g2,
                                           512 * ic + off:512 * (ic + 1)],
                                        start=True, stop=True,
                                        tile_position=(band, 64 * cg),
                                    )
                        # one exp for both heads via a [128, 2, w] strided AP
                        e_big = epool.tile([P, 2, 512], EXP_DT, tag="e",
                                           name=f"e_{g2}_{ic}_{jb}")
                        nc.scalar.activation(
                            out=e_big[:, :, off:],
                            in_=s_big.rearrange("p (h2 i) -> p h2 i", h2=2)[:, :, off:],
                            func=mybir.ActivationFunctionType.Exp,
                        )
                        if P * jb >= 512 * ic:  # diagonal triangle mask
                            for hh in range(2):
                                nc.gpsimd.affine_select(
                                    out=e_big[:, hh, off:off + P],
                                    in_=e_big[:, hh, off:off + P],
                                    compare_op=mybir.AluOpType.is_ge,
                                    fill=0.0, base=0, channel_multiplier=-1,
                                    pattern=[[1, P]],
                                )
                        for hh in range(2):
                            h = 2 * g2 + hh
                            nc.tensor.matmul(
                                o_ps[hh][:, off:], v_aug[:, jb, h, :],
                                e_big[:, hh, off:],
                                start=(jb == 0), stop=(jb == n_jb - 1),
                            )
                    for hh in range(2):
                        h = 2 * g2 + hh
                        idx = 4 * hh + ic
                        o_u = upool.tile([D + 1, 512], f32, tag="ou",
                                         name=f"ou_{h}_{ic}")
                        nc.vector.tensor_copy(o_u, o_ps[hh])
                        nc.sync.dma_start(out=S_pair[idx:idx + 1, :], in_=o_u[D:D + 1, :])
                        o_us[idx] = o_u
                R_pair = spool.tile([8, 512], f32, tag="R", name=f"R_{g2}")
                nc.vector.reciprocal(R_pair, S_pair)
                for hh in range(2):
                    h = 2 * g2 + hh
                    for ic in range(TCH):
                        idx = 4 * hh + ic
                        rrow = bpool.tile([1, 512], f32, tag="rrow", name=f"rr_{h}_{ic}")
                        nc.sync.dma_start(out=rrow, in_=R_pair[idx:idx + 1, :])
                        rb = bpool.tile([D, 512], f32, tag="rb", name=f"rb_{h}_{ic}")
                        nc.gpsimd.partition_broadcast(rb, rrow[0:1, :])
                        nc.vector.tensor_mul(
                            out=oT[64 * (h % 2):64 * (h % 2) + D, h // 2,
                                   512 * ic:512 * (ic + 1)],
                            in0=o_us[idx][0:D, :],
                            in1=rb,
                        )

        # ---------------- phase 3: output projection ----------------
        with (
            tc.tile_pool(name="ypool", bufs=4) as ypool,
            tc.tile_pool(name="ps_y", bufs=3, space="PSUM") as ps_y,
        ):
            for tt in range(T // P):
                for mc in range(C // 512):
                    y_ps = ps_y.tile([P, 512], f32, tag="y", name=f"y_{tt}_{mc}")
                    for nt in range(NT):
                        nc.tensor.matmul(
                            y_ps,
                            oT[:, nt, P * tt:P * (tt + 1)],
                            wp_sb[:, nt, 512 * mc:512 * (mc + 1)],
                            start=(nt == 0), stop=(nt == NT - 1),
                        )
                    y_sb = ypool.tile([P, 512], f32, tag="ysb", name=f"ysb_{tt}_{mc}")
                    nc.vector.tensor_copy(out=y_sb, in_=y_ps)
                    nc.sync.dma_start(
                        out=y.ap()[P * tt:P * (tt + 1), 512 * mc:512 * (mc + 1)],
                        in_=y_sb,
                    )


_NC_CACHE = {}


def _get_nc():
    if "nc" not in _NC_CACHE:
        nc = bacc.Bacc("TRN2", debug=False, num_devices=8)
        build_attention(nc)
        nc.compile()
        _NC_CACHE["nc"] = nc
    return _NC_CACHE["nc"]


def kernel(x, W_attn, b_attn, W_proj, b_proj):
    x = np.asarray(x, dtype=np.float32)
    W_attn = np.asarray(W_attn, dtype=np.float32)
    b_attn = np.asarray(b_attn, dtype=np.float32)
    W_proj = np.asarray(W_proj, dtype=np.float32)
    b_proj = np.asarray(b_proj, dtype=np.float32)

    import ml_dtypes
    mm_np = (np.float32 if MM_DT in (f32, f32r) else ml_dtypes.bfloat16)

    scale = 1.0 / np.sqrt(np.float32(D))
    in_maps = []
    for core in range(8):
        b, g = divmod(core, 2)
        cols = slice(G * g, G * (g + 1))
        bqs = (b_attn[0:C][cols] * scale).reshape(NT, 2, D).transpose(1, 2, 0).reshape(P, NT)
        bks = b_attn[C:2 * C][cols].reshape(NT, 2, D).transpose(1, 2, 0).reshape(P, NT)
        in_maps.append({
            "xT": np.ascontiguousarray(x[b].T).astype(mm_np),
            "wq": np.ascontiguousarray(W_attn[:, 0:C][:, cols] * scale).astype(mm_np),
            "wk": np.ascontiguousarray(W_attn[:, C:2 * C][:, cols]).astype(mm_np),
            "wv": np.ascontiguousarray(W_attn[:, 2 * C:3 * C][:, cols]).astype(mm_np),
            "wp": np.ascontiguousarray(W_proj[G * g:G * (g + 1), :]).astype(mm_np),
            "bq": np.ascontiguousarray(bqs),
            "bk": np.ascontiguousarray(bks),
        })

    res = run_bass_kernel_spmd(_get_nc(), in_maps, core_ids=list(range(8)))

    correction = b_attn[2 * C:3 * C] @ W_proj + b_proj  # [C]
    out = np.empty((B, T, C), dtype=np.float32)
    for b in range(B):
        out[b] = res.results[2 * b]["y"] + res.results[2 * b + 1]["y"] + correction
    return out


# revision 15
# speedup vs baseline: 1.5552x; 1.5552x over previous
"""Causal self-attention on 8 trn2 NeuronCores.

Full inputs in, full output out. Sharding: data-parallel over batch (B=4),
tensor-parallel over head groups (16 heads -> 2 groups of 8). core = 2*b + g.

Per-core math (T=2048, C=1024, 8 heads, D=64, group channels G=512):
  qT/kT: [64*(h%2)+d, h//2, t] layout so scores need no transposes
  scoresT[j,i] = sum_d kT[d,j] qT[d,i]   (q pre-scaled by 1/sqrt(D) on host)
  softmax without max-subtraction (scores ~ N(0,1) by construction; exp is
  exactly shift-invariant so this matches the reference softmax)
  expT row sums come free from an all-ones 65th column appended to V
  causal mask via affine_select (fill 0 post-exp) on diagonal blocks only
  out_T[d,i] = sum_j v[j,d] expT[j,i]; normalize by 1/sums; y = oT.T @ Wp

Host gather: y[b] = part[2b] + part[2b+1] + b_attn_v @ W_proj + b_proj
(q/k biases are added on-device; the v bias commutes through softmax).

All matmul inputs are float32r (hardware rounds fp32 on read; measured
~1.5e-4 rel err per K=1024 matmul, at full 1 cyc/row PE rate for N>=256).
"""

import numpy as np
from contextlib import ExitStack

import concourse.bass as bass
import concourse.tile as tile
from concourse import bacc, mybir
from concourse.bass_utils import run_bass_kernel_spmd

P = 128
B, T, C, H = 4, 2048, 1024, 16
D = 64
HG = 8          # heads per core
G = HG * D      # 512 head channels per core
CT = C // P     # 8 contraction tiles
TCH = T // 512  # 4 chunks of 512 tokens
NT = G // P     # 4 tiles of head channels

f32 = mybir.dt.float32
f32r = mybir.dt.float32r
bf16 = mybir.dt.bfloat16
MM_DT = bf16    # dtype of all matmul inputs (bf16: FWL weight loads, 1 cyc/row)
EXP_DT = MM_DT  # dtype of exp(scores) tiles (step-4 rhs)


def build_attention(nc: bass.Bass):
    xT = nc.dram_tensor("xT", [C, T], MM_DT, kind="ExternalInput")
    wq = nc.dram_tensor("wq", [C, G], MM_DT, kind="ExternalInput")
    wk = nc.dram_tensor("wk", [C, G], MM_DT, kind="ExternalInput")
    wv = nc.dram_tensor("wv", [C, G], MM_DT, kind="ExternalInput")
    wp = nc.dram_tensor("wp", [G, C], MM_DT, kind="ExternalInput")
    bq = nc.dram_tensor("bq", [P, NT], f32, kind="ExternalInput")
    bk = nc.dram_tensor("bk", [P, NT], f32, kind="ExternalInput")
    y = nc.dram_tensor("y", [T, C], f32, kind="ExternalOutput")

    with tile.TileContext(nc) as tc, ExitStack() as ctx:
        persist = ctx.enter_context(tc.tile_pool(name="persist", bufs=1))
        qT = persist.tile([P, NT, T], MM_DT)
        kT = persist.tile([P, NT, T], MM_DT)
        v_aug = persist.tile([P, T // P, HG, D + 1], MM_DT)
        wp_sb = persist.tile([P, NT, C], MM_DT)
        bq_sb = persist.tile([P, NT], f32)
        bk_sb = persist.tile([P, NT], f32)

        for nt in range(NT):
            nc.sync.dma_start(out=wp_sb[:, nt, :], in_=wp.ap()[P * nt:P * (nt + 1), :])
        nc.sync.dma_start(out=bq_sb, in_=bq.ap())
        nc.sync.dma_start(out=bk_sb, in_=bk.ap())
        ones_col = persist.tile([P, 1], f32)
        nc.vector.memset(ones_col, 1.0)
        nc.vector.tensor_copy(
            out=v_aug[:, :, :, D:D + 1],
            in_=ones_col.to_broadcast([P, T // P, HG, 1]),
        )

        # ---------------- phase 1: QKV projections ----------------
        with (
            tc.tile_pool(name="wpool", bufs=1) as wpool,
            tc.tile_pool(name="xpool", bufs=10) as xpool,
            tc.tile_pool(name="ps1", bufs=3, space="PSUM") as ps1,
        ):
            wq_sb = wpool.tile([P, CT, G], MM_DT, tag="wq")
            wk_sb = wpool.tile([P, CT, G], MM_DT, tag="wk")
            wv_sb = wpool.tile([P, CT, G], MM_DT, tag="wv")
            for ct in range(CT):
                nc.sync.dma_start(out=wq_sb[:, ct, :], in_=wq.ap()[P * ct:P * (ct + 1), :])
                nc.sync.dma_start(out=wk_sb[:, ct, :], in_=wk.ap()[P * ct:P * (ct + 1), :])
                nc.sync.dma_start(out=wv_sb[:, ct, :], in_=wv.ap()[P * ct:P * (ct + 1), :])

            for tch in range(TCH):
                xts = []
                for ct in range(CT):
                    xt = xpool.tile([P, 512], MM_DT, tag="xt", name=f"xt_{tch}_{ct}")
                    nc.sync.dma_start(
                        out=xt, in_=xT.ap()[P * ct:P * (ct + 1), 512 * tch:512 * (tch + 1)]
                    )
                    xts.append(xt)
                # qT, kT:  psum[j,t] += wq[c,j].T @ xT[c,t]
                for w_sb, b_sb, dstT in ((wq_sb, bq_sb, qT), (wk_sb, bk_sb, kT)):
                    for jt in range(NT):
                        ps = ps1.tile([P, 512], f32, tag="pqk", name=f"pqk_{tch}_{jt}")
                        for ct in range(CT):
                            nc.tensor.matmul(
                                ps, w_sb[:, ct, P * jt:P * (jt + 1)], xts[ct],
                                start=(ct == 0), stop=(ct == CT - 1),
                            )
                        nc.vector.tensor_scalar_add(
                            out=dstT[:, jt, 512 * tch:512 * (tch + 1)],
                            in0=ps, scalar1=b_sb[:, jt:jt + 1],
                        )
                # v: psum[t,jv] += xT[c,t-tile].T @ wv[c,jv]
                for tt4 in range(4):
                    tt = 4 * tch + tt4
                    ps = ps1.tile([P, 512], f32, tag="pv", name=f"pv_{tt}")
                    for ct in range(CT):
                        nc.tensor.matmul(
                            ps, xts[ct][:, P * tt4:P * (tt4 + 1)], wv_sb[:, ct, :],
                            start=(ct == 0), stop=(ct == CT - 1),
                        )
                    nc.vector.tensor_copy(
                        out=v_aug[:, tt, :, 0:D],
                        in_=ps.rearrange("p (h d) -> p h d", h=HG),
                    )

        # ---------------- phase 2: attention ----------------
        # Packed scores: 2 heads concurrently on the PE via tile_position
        # 4-tile row+col packing (head h occupies array rows 64*(h%2)..+64).
        # Variable-width trapezoid tiles: block (jb, ic) only covers queries
        # i >= 128*jb, so all of scores/exp/out use columns [off:] with
        # off = max(0, 128*jb - 512*ic). Diagonal 128-wide triangle masked
        # with a single [128,128] affine_select.
        opool = ctx.enter_context(tc.tile_pool(name="opool", bufs=1))
        oT = opool.tile([P, NT, T], MM_DT)
        with (
            tc.tile_pool(name="epool", bufs=6) as epool,
            tc.tile_pool(name="upool", bufs=10) as upool,
            tc.tile_pool(name="spool", bufs=2) as spool,
            tc.tile_pool(name="bpool", bufs=3) as bpool,
            tc.tile_pool(name="ps_s", bufs=2, space="PSUM") as ps_s,
            tc.tile_pool(name="ps_o", bufs=3, space="PSUM") as ps_o,
        ):
            for g2 in range(HG // 2):  # head pairs share nt = g2
                S_pair = spool.tile([8, 512], f32, tag="S", name=f"S_{g2}")
                o_us = {}
                for ic in range(TCH):
                    o_ps = {}
                    for hh in range(2):
                        o_ps[hh] = ps_o.tile([D + 1, 512], f32, tag="o",
                                             name=f"ops_{2 * g2 + hh}_{ic}")
                    n_jb = 4 * ic + 4
                    for jb in range(n_jb):
                        off = max(0, P * jb - 512 * ic)
                        w = 512 - off
                        # both heads' scores in one 2-bank psum tile:
                        # cols [off:512] = h0 (from array rows 0:64),
                        # cols [512+off:1024] = h1 (rows 64:128)
                        s_big = ps_s.tile([P, 1024], f32, tag="s",
                                          name=f"sps_{g2}_{ic}_{jb}")
                        for hh in range(2):
                            band = 64 * hh
                            nc.tensor.matmul(
                                s_big[:, 512 * hh + off:512 * (hh + 1)],
                                kT[band:band + D, g2, P * jb:P * (jb + 1)],
                                qT[band:band + D, g2, 512 * ic + off:512 * (ic + 1)],
                                start=True, stop=True,
                            )
                        # one exp for both heads via a [128, 2, w] strided AP
                        e_big = epool.tile([P, 2, 512], EXP_DT, tag="e",
                                           name=f"e_{g2}_{ic}_{jb}")
                        nc.scalar.activation(
                            out=e_big[:, :, off:],
                            in_=s_big.rearrange("p (h2 i) -> p h2 i", h2=2)[:, :, off:],
                            func=mybir.ActivationFunctionType.Exp,
                        )
                        if P * jb >= 512 * ic:  # diagonal triangle mask
                            for hh in range(2):
                                nc.gpsimd.affine_select(
                                    out=e_big[:, hh, off:off + P],
                                    in_=e_big[:, hh, off:off + P],
                                    compare_op=mybir.AluOpType.is_ge,
                                    fill=0.0, base=0, channel_multiplier=-1,
                                    pattern=[[1, P]],
                                )
                        for hh in range(2):
                            h = 2 * g2 + hh
                            nc.tensor.matmul(
                                o_ps[hh][:, off:], v_aug[:, jb, h, :],
                                e_big[:, hh, off:],
                                start=(jb == 0), stop=(jb == n_jb - 1),
                            )
                    for hh in range(2):
                        h = 2 * g2 + hh
                        idx = 4 * hh + ic
                        o_u = upool.tile([D + 1, 512], f32, tag="ou",
                                         name=f"ou_{h}_{ic}")
                        nc.vector.tensor_copy(o_u, o_ps[hh])
                        nc.sync.dma_start(out=S_pair[idx:idx + 1, :], in_=o_u[D:D + 1, :])
                        o_us[idx] = o_u
                R_pair = spool.tile([8, 512], f32, tag="R", name=f"R_{g2}")
                nc.vector.reciprocal(R_pair, S_pair)
                for hh in range(2):
                    h = 2 * g2 + hh
                    for ic in range(TCH):
                        idx = 4 * hh + ic
                        rrow = bpool.tile([1, 512], f32, tag="rrow", name=f"rr_{h}_{ic}")
                        nc.sync.dma_start(out=rrow, in_=R_pair[idx:idx + 1, :])
                        rb = bpool.tile([D, 512], f32, tag="rb", name=f"rb_{h}_{ic}")
                        nc.gpsimd.partition_broadcast(rb, rrow[0:1, :])
                        nc.vector.tensor_mul(
                            out=oT[64 * (h % 2):64 * (h % 2) + D, h // 2,
                                   512 * ic:512 * (ic + 1)],
                            in0=o_us[idx][0:D, :],
                            in1=rb,
                        )

        # ---------------- phase 3: output projection ----------------
        with (
            tc.tile_pool(name="ypool", bufs=4) as ypool,
            tc.tile_pool(name="ps_y", bufs=3, space="PSUM") as ps_y,
        ):
            for tt in range(T // P):
                for mc in range(C // 512):
                    y_ps = ps_y.tile([P, 512], f32, tag="y", name=f"y_{tt}_{mc}")
                    for nt in range(NT):
                        nc.tensor.matmul(
                            y_ps,
                            oT[:, nt, P * tt:P * (tt + 1)],
                            wp_sb[:, nt, 512 * mc:512 * (mc + 1)],
                            start=(nt == 0), stop=(nt == NT - 1),
                        )
                    y_sb = ypool.tile([P, 512], f32, tag="ysb", name=f"ysb_{tt}_{mc}")
                    nc.vector.tensor_copy(out=y_sb, in_=y_ps)
                    nc.sync.dma_start(
                        out=y.ap()[P * tt:P * (tt + 1), 512 * mc:512 * (mc + 1)],
                        in_=y_sb,
                    )


_NC_CACHE = {}


def _get_nc():
    if "nc" not in _NC_CACHE:
        nc = bacc.Bacc("TRN2", debug=False, num_devices=8)
        build_attention(nc)
        nc.compile()
        _NC_CACHE["nc"] = nc
    return _NC_CACHE["nc"]


def kernel(x, W_attn, b_attn, W_proj, b_proj):
    x = np.asarray(x, dtype=np.float32)
    W_attn = np.asarray(W_attn, dtype=np.float32)
    b_attn = np.asarray(b_attn, dtype=np.float32)
    W_proj = np.asarray(W_proj, dtype=np.float32)
    b_proj = np.asarray(b_proj, dtype=np.float32)

    import ml_dtypes
    mm_np = (np.float32 if MM_DT in (f32, f32r) else ml_dtypes.bfloat16)

    scale = 1.0 / np.sqrt(np.float32(D))
    in_maps = []
    for core in range(8):
        b, g = divmod(core, 2)
        cols = slice(G * g, G * (g + 1))
        bqs = (b_attn[0:C][cols] * scale).reshape(NT, 2, D).transpose(1, 2, 0).reshape(P, NT)
        bks = b_attn[C:2 * C][cols].reshape(NT, 2, D).transpose(1, 2, 0).reshape(P, NT)
        in_maps.append({
            "xT": np.ascontiguousarray(x[b].T).astype(mm_np),
            "wq": np.ascontiguousarray(W_attn[:, 0:C][:, cols] * scale).astype(mm_np),
            "wk": np.ascontiguousarray(W_attn[:, C:2 * C][:, cols]).astype(mm_np),
            "wv": np.ascontiguousarray(W_attn[:, 2 * C:3 * C][:, cols]).astype(mm_np),
            "wp": np.ascontiguousarray(W_proj[G * g:G * (g + 1), :]).astype(mm_np),
            "bq": np.ascontiguousarray(bqs),
            "bk": np.ascontiguousarray(bks),
        })

    res = run_bass_kernel_spmd(_get_nc(), in_maps, core_ids=list(range(8)))

    correction = b_attn[2 * C:3 * C] @ W_proj + b_proj  # [C]
    out = np.empty((B, T, C), dtype=np.float32)
    for b in range(B):
        out[b] = res.results[2 * b]["y"] + res.results[2 * b + 1]["y"] + correction
    return out
